# revision 26
# baseline (speedup 1.0000x reference)
"""Self-contained Trainium2 Bass kernel for a batched (time-stepped) GAT layer.

Problem: x [N=20000, T=8, F=128], edge_index [2, E=320000] (+self loops),
W [128, 256] (4 heads x 64), att_src/att_dst [4, 64], bias [64].
Per time step: GATConv (concat=False -> head mean) with softmax attention.
Output: [N, T, 64] f32.

Sharding: data-parallel over the T=8 time steps across 8 NeuronCores
(each step independent given shared weights; weights replicated).

Per-core algorithm (one time step):
  Phase 1 (dense): h = x_t @ W_aug (W augmented with 8 columns so the same
    matmul yields per-node a_src/a_dst attention logits). h is written to an
    HBM side array 'hext' ([N, 384] bf16 rows: 256 bf16 h | 4 f32 a_src | pad)
    and per-node aux rows 'aux' ([N+pad, 128] fp16: 4 a_dst | dst_local | pad).
  Phase 2 (edges, sorted by destination on host, chunked 128/dst-tile):
    - dma_gather hext rows by src  -> h[src], a_src[src]
    - dma_gather aux rows by dst   -> a_dst[dst], dst_local (pad edges hit a
      dummy aux row with a_dst=-1000 so exp(alpha) == 0)
    - alpha = leaky_relu(a_src+a_dst); ex = exp(alpha)   (softmax max-shift is
      unnecessary: |alpha| <= ~10, exp stays in f32 range; softmax invariant)
    - msg = h[src] * ex (broadcast per head) ++ ex columns
    - one-hot(dst_local) matmul accumulates segment sums into PSUM:
      numerator [128, 256] and denominators [128, 4] in one [128, 260] matmul
    - out = (numerator / denom).mean(heads) + bias
"""

import numpy as np
import ml_dtypes
from contextlib import ExitStack

import concourse.bass as bass
import concourse.bacc as bacc
import concourse.mybir as mybir
import concourse.tile as tile
from concourse import library_config
from concourse.bass_utils import run_bass_kernel_spmd

F32 = mybir.dt.float32
BF16 = mybir.dt.bfloat16
FP16 = mybir.dt.float16
I16 = mybir.dt.int16

P = 128


class GatConfig:
    def __init__(self, n_nodes, in_dim, heads, d_model, neg_slope):
        self.n_nodes = n_nodes
        self.in_dim = in_dim
        self.heads = heads
        self.d_model = d_model
        self.hc = heads * d_model
        self.neg_slope = neg_slope
        self.n_tiles = (n_nodes + P - 1) // P
        self.n_pad = self.n_tiles * P
        self.dummy_row = self.n_pad            # aux dummy row index
        self.aux_rows = self.n_pad + P
        self.hext_w = self.hc + P              # bf16 cols: hc h | 8 a_src | pad
        self.aux_w = P                         # fp16 cols: heads a_dst | 1 dstloc
        self.mm_w = self.hc + heads            # matmul rhs width (msg | ex)


CFG = GatConfig(n_nodes=20000, in_dim=128, heads=4, d_model=64, neg_slope=0.2)
T_STEPS = 8
N_CORES = 8


def preprocess_edges(cfg, edge_index):
    """Sort (edges + self loops) by destination, pad each 128-node dst tile's
    edge list to a multiple of 128, and produce wrapped int16 gather indices.

    Returns (g1_wrapped, g2_wrapped, chunks_per_tile).
    g1: source-node index per edge slot (pad slots -> 0, harmless: ex==0).
    g2: dst-node index per edge slot (pad slots -> dummy aux row).
    Wrapped layout: flat slot j lives at [j % 16, j // 16], replicated to
    128 partitions (8 copies of the 16-partition group) as HW requires.
    """
    n = cfg.n_nodes
    loops = np.arange(n, dtype=np.int64)
    src = np.concatenate([np.asarray(edge_index[0], dtype=np.int64), loops])
    dst = np.concatenate([np.asarray(edge_index[1], dtype=np.int64), loops])
    order = np.argsort(dst, kind="stable")
    src_s = src[order]
    dst_s = dst[order]
    counts = np.bincount(dst_s // P, minlength=cfg.n_tiles)
    g1_parts, g2_parts, chunks = [], [], []
    pos = 0
    for m in range(cfg.n_tiles):
        length = int(counts[m])
        lpad = max(P, ((length + P - 1) // P) * P)
        g1 = np.zeros(lpad, np.int16)
        g2 = np.full(lpad, cfg.dummy_row, np.int16)
        g1[:length] = src_s[pos : pos + length]
        g2[:length] = dst_s[pos : pos + length]
        g1_parts.append(g1)
        g2_parts.append(g2)
        chunks.append(lpad // P)
        pos += length
    assert pos == src_s.size

    def wrap(flat):
        w = flat.reshape(-1, 16).T.copy()       # [16, E_pad/16]
        return np.tile(w, (8, 1)).copy()        # [128, E_pad/16]

    g1_all = np.concatenate(g1_parts)
    g2_all = np.concatenate(g2_parts)
    return wrap(g1_all), wrap(g2_all), chunks


def build_consts(cfg, W, att_src, att_dst, bias):
    """Host-side constant tensors shared by all cores."""
    W = np.asarray(W, np.float32)
    att_src = np.asarray(att_src, np.float32)
    att_dst = np.asarray(att_dst, np.float32)
    bias = np.asarray(bias, np.float32)
    Wr = W.reshape(cfg.in_dim, cfg.heads, cfg.d_model)
    a_src_cols = np.einsum("fhc,hc->fh", Wr, att_src)
    a_dst_cols = np.einsum("fhc,hc->fh", Wr, att_dst)
    waug = np.concatenate([W, a_src_cols, a_dst_cols], axis=1)  # [F, hc+2H]
    biasrep = np.tile(bias[None, :], (P, 1)).astype(np.float32)
    t2row = np.tile(np.arange(P, dtype=np.float16)[None, :], (P, 1))
    pcol = np.arange(P, dtype=np.float16)[:, None].copy()
    ident = np.eye(P, dtype=np.float32)
    return {
        "waug": np.ascontiguousarray(waug, np.float32),
        "biasrep": biasrep,
        "t2row": t2row,
        "pcol": pcol,
        "ident": ident,
    }


def build_nc(cfg, chunks, e16, debug=False, num_devices=N_CORES):
    """Build the full Bass program (SPMD: identical across cores)."""
    nc = bacc.Bacc(
        "TRN2", target_bir_lowering=False, debug=debug, num_devices=num_devices
    )
    heads, hc = cfg.heads, cfg.hc
    naug = hc + 2 * heads

    xt = nc.dram_tensor("xt", [cfg.n_nodes, cfg.in_dim], F32, kind="ExternalInput")
    waug = nc.dram_tensor("waug", [cfg.in_dim, naug], F32, kind="ExternalInput")
    biasrep = nc.dram_tensor("biasrep", [P, cfg.d_model], F32, kind="ExternalInput")
    t2row = nc.dram_tensor("t2row", [P, P], FP16, kind="ExternalInput")
    pcol = nc.dram_tensor("pcol", [P, 1], FP16, kind="ExternalInput")
    ident = nc.dram_tensor("ident", [P, P], F32, kind="ExternalInput")
    g1 = nc.dram_tensor("g1", [P, e16], I16, kind="ExternalInput")
    g2 = nc.dram_tensor("g2", [P, e16], I16, kind="ExternalInput")
    hext = nc.dram_tensor("hext", [cfg.n_pad, cfg.hext_w], BF16, kind="Internal")
    aux = nc.dram_tensor("aux", [cfg.aux_rows, cfg.aux_w], FP16, kind="Internal")
    out = nc.dram_tensor("out", [cfg.n_nodes, cfg.d_model], F32, kind="ExternalOutput")

    with tile.TileContext(nc) as tc, ExitStack() as ctx:
        nc.gpsimd.load_library(library_config.mlp)
        tc.no_sync_barrier()

        consts = ctx.enter_context(tc.tile_pool(name="consts", bufs=1))
        waug_t = consts.tile([P, naug], F32)
        nc.sync.dma_start(waug_t[:], waug[:, :])
        bias_t = consts.tile([P, cfg.d_model], F32)
        nc.sync.dma_start(bias_t[:], biasrep[:, :])
        t2_t = consts.tile([P, P], FP16)
        nc.sync.dma_start(t2_t[:], t2row[:, :])
        pcol_t = consts.tile([P, 1], FP16)
        nc.sync.dma_start(pcol_t[:], pcol[:, :])
        id_t = consts.tile([P, P], F32)
        nc.sync.dma_start(id_t[:], ident[:, :])

        # ---------------- phase 1: dense h + logits ----------------
        h_scope = nc.enter_named_scope("h_phase", False)[0]
        xpool = ctx.enter_context(tc.tile_pool(name="x", bufs=3))
        stpool = ctx.enter_context(tc.tile_pool(name="stage", bufs=3))
        ps_tr = ctx.enter_context(tc.tile_pool(name="ps_tr", bufs=2, space="PSUM"))
        ps_h = ctx.enter_context(tc.tile_pool(name="ps_h", bufs=2, space="PSUM"))

        for m in range(cfg.n_tiles):
            n0 = m * P
            nrows = min(P, cfg.n_nodes - n0)
            xtile = xpool.tile([P, cfg.in_dim], F32, tag="xtile")
            if nrows < P:
                nc.vector.memset(xtile[:], 0.0)
            nc.sync.dma_start(xtile[:nrows, :], xt[n0 : n0 + nrows, :])
            ptr = ps_tr.tile([P, P], F32)
            nc.tensor.transpose(ptr[:], xtile[:], id_t[:])
            xT = xpool.tile([P, P], F32, tag="xT")
            nc.vector.tensor_copy(xT[:], ptr[:])
            ph = ps_h.tile([P, naug], F32)
            nc.tensor.matmul(ph[:], xT[:], waug_t[:], start=True, stop=True)

            stage = stpool.tile([P, cfg.hext_w], BF16, tag="stage")
            nc.vector.memset(stage[:, hc + 8 :], 0.0)
            nc.vector.tensor_copy(stage[:, 0:hc], ph[:, 0:hc])
            nc.vector.tensor_copy(
                stage[:, hc : hc + 8].bitcast(F32), ph[:, hc : hc + heads]
            )
            nc.sync.dma_start(hext[n0 : n0 + P, :], stage[:])

            astage = stpool.tile([P, cfg.aux_w], FP16, tag="astage")
            nc.vector.memset(astage[:], 0.0)
            nc.vector.tensor_copy(astage[:, 0:heads], ph[:, hc + heads : naug])
            nc.vector.tensor_copy(astage[:, heads : heads + 1], pcol_t[:])
            nc.sync.dma_start(aux[n0 : n0 + P, :], astage[:])

        # dummy aux rows for padded edge slots: a_dst = -1000 => ex == 0
        dstage = stpool.tile([P, cfg.aux_w], FP16, tag="astage")
        nc.vector.memset(dstage[:], -1000.0)
        nc.sync.dma_start(aux[cfg.n_pad : cfg.n_pad + P, :], dstage[:])

        nc.leave_named_scope("h_phase", h_scope, False)
        tc.strict_bb_all_engine_barrier()

        # ---------------- phase 2: edge message passing ----------------
        e_scope = nc.enter_named_scope("edge_phase", False)[0]
        idxpool = ctx.enter_context(tc.tile_pool(name="idx", bufs=1))
        g1s = idxpool.tile([P, e16], I16)
        nc.sync.dma_start(g1s[:], g1[:, :])
        g2s = idxpool.tile([P, e16], I16)
        nc.sync.dma_start(g2s[:], g2[:, :])

        max_ch = max(chunks)
        hpool = ctx.enter_context(tc.tile_pool(name="hrow", bufs=2))
        apool = ctx.enter_context(tc.tile_pool(name="arow", bufs=2))
        mpool = ctx.enter_context(tc.tile_pool(name="msg", bufs=2))
        ohpool = ctx.enter_context(tc.tile_pool(name="oh", bufs=4))
        spool = ctx.enter_context(tc.tile_pool(name="small", bufs=2))
        ps_e = ctx.enter_context(tc.tile_pool(name="ps_e", bufs=2, space="PSUM"))

        # Split gathers: SWDGE descriptor-ring carveout holds ~256 descs per
        # partition; one gather emits num_idxs/16 descs per partition, so keep
        # each call at <= GMAX indices.
        GMAX_CH = 8  # 1024 indices / call
        sub_lens = set()
        for nch in set(chunks):
            for c0 in range(0, nch, GMAX_CH):
                sub_lens.add(min(GMAX_CH, nch - c0) * P)
        lregs = {l: nc.gpsimd.to_reg(l) for l in sorted(sub_lens)}

        off = 0
        for m in range(cfg.n_tiles):
            nch = chunks[m]
            hrow = hpool.tile([P, max_ch, cfg.hext_w], BF16)
            arow = apool.tile([P, max_ch, cfg.aux_w], FP16)
            for c0 in range(0, nch, GMAX_CH):
                cc = min(GMAX_CH, nch - c0)
                ll = cc * P
                o0 = off + c0 * P
                nc.gpsimd.dma_gather(
                    hrow[:, c0 : c0 + cc, :],
                    hext[:, :],
                    g1s[:, o0 // 16 : (o0 + ll) // 16],
                    ll,
                    lregs[ll],
                    cfg.hext_w,
                    queue_num=0,
                )
                nc.gpsimd.dma_gather(
                    arow[:, c0 : c0 + cc, :],
                    aux[:, :],
                    g2s[:, o0 // 16 : (o0 + ll) // 16],
                    ll,
                    lregs[ll],
                    cfg.aux_w,
                    queue_num=0,
                )
            off += nch * P

            # alpha = leaky_relu(a_src[src] + a_dst[dst]); ex = exp(alpha)
            alpha = spool.tile([P, max_ch, heads], F32, tag="alpha")
            nc.vector.tensor_add(
                alpha[:, 0:nch, :],
                hrow[:, 0:nch, hc : hc + 8].bitcast(F32),
                arow[:, 0:nch, 0:heads],
            )
            lrt = spool.tile([P, max_ch, heads], F32, tag="lrt")
            nc.vector.tensor_scalar_mul(lrt[:, 0:nch, :], alpha[:, 0:nch, :], cfg.neg_slope)
            nc.vector.tensor_max(alpha[:, 0:nch, :], alpha[:, 0:nch, :], lrt[:, 0:nch, :])
            exb = spool.tile([P, max_ch, heads], BF16, tag="exb")
            nc.scalar.activation(
                exb[:, 0:nch, :], alpha[:, 0:nch, :], mybir.ActivationFunctionType.Exp
            )

            # weighted messages: msg[:, :, :hc] = h * ex (per head), ++ ex cols
            msg = mpool.tile([P, max_ch, cfg.mm_w], BF16)
            nc.vector.tensor_tensor(
                msg[:, 0:nch, 0:hc].rearrange("p n (h c) -> p n h c", h=heads),
                hrow[:, 0:nch, 0:hc].rearrange("p n (h c) -> p n h c", h=heads),
                exb[:, 0:nch, :].broadcast_to((P, nch, heads, cfg.d_model)),
                op=mybir.AluOpType.mult,
            )
            nc.vector.tensor_copy(msg[:, 0:nch, hc : cfg.mm_w], exb[:, 0:nch, :])

            # segment sums via one-hot matmul into PSUM
            dl32 = spool.tile([P, max_ch, 1], F32, tag="dl32")
            nc.vector.tensor_copy(dl32[:, 0:nch, :], arow[:, 0:nch, heads : heads + 1])
            pe = ps_e.tile([P, cfg.mm_w], F32)
            for ch in range(nch):
                oh = ohpool.tile([P, P], BF16)
                nc.vector.tensor_scalar(
                    oh[:],
                    t2_t[:],
                    dl32[:, ch, :],
                    None,
                    op0=mybir.AluOpType.is_equal,
                )
                nc.tensor.matmul(
                    pe[:],
                    oh[:],
                    msg[:, ch, :],
                    start=(ch == 0),
                    stop=(ch == nch - 1),
                )

            # out = (numerator / denom).mean(heads) + bias
            r = spool.tile([P, heads], F32, tag="r")
            nc.vector.reciprocal(r[:], pe[:, hc : cfg.mm_w])
            nc.vector.tensor_scalar_mul(r[:], r[:], 1.0 / heads)
            wm = spool.tile([P, heads, cfg.d_model], F32, tag="wm")
            nc.vector.tensor_tensor(
                wm[:],
                pe[:, 0:hc].rearrange("p (h c) -> p h c", h=heads),
                r[:].broadcast_to((P, heads, cfg.d_model)),
                op=mybir.AluOpType.mult,
            )
            onode = spool.tile([P, cfg.d_model], F32, tag="onode")
            nc.vector.tensor_reduce(
                onode[:],
                wm[:].rearrange("p h c -> p c h"),
                axis=mybir.AxisListType.X,
                op=mybir.AluOpType.add,
            )
            nc.vector.tensor_add(onode[:], onode[:], bias_t[:])
            n0 = m * P
            nrows = min(P, cfg.n_nodes - n0)
            nc.sync.dma_start(out[n0 : n0 + nrows, :], onode[:nrows, :])

        nc.leave_named_scope("edge_phase", e_scope, False)

    nc.compile()
    return nc


_CACHE = {}


def _prepare(x, edge_index, W, att_src, att_dst, bias):
    cfg = CFG
    x = np.asarray(x, np.float32)
    key = hash(np.asarray(edge_index).tobytes())
    if key not in _CACHE:
        g1w, g2w, chunks = preprocess_edges(cfg, edge_index)
        nc = build_nc(cfg, chunks, g1w.shape[1], debug=False, num_devices=N_CORES)
        _CACHE.clear()
        _CACHE[key] = (nc, g1w, g2w)
    nc, g1w, g2w = _CACHE[key]
    consts = build_consts(cfg, W, att_src, att_dst, bias)
    in_maps = []
    for t in range(T_STEPS):
        in_maps.append(
            {
                "xt": np.ascontiguousarray(x[:, t, :]),
                "g1": g1w,
                "g2": g2w,
                **consts,
            }
        )
    return nc, in_maps


def kernel(x, edge_index, W, att_src, att_dst, bias):
    nc, in_maps = _prepare(x, edge_index, W, att_src, att_dst, bias)
    res = run_bass_kernel_spmd(nc, in_maps, core_ids=list(range(N_CORES)))
    outs = [res.results[t]["out"] for t in range(T_STEPS)]
    return np.stack(outs, axis=1)  # [N, T, C]


def kernel_profiled(x, edge_index, W, att_src, att_dst, bias):
    """Run with NTFF tracing; returns (output, exec_time_ns, results obj)."""
    nc, in_maps = _prepare(x, edge_index, W, att_src, att_dst, bias)
    res = run_bass_kernel_spmd(
        nc, in_maps, core_ids=list(range(N_CORES)), trace=True
    )
    outs = [res.results[t]["out"] for t in range(T_STEPS)]
    return np.stack(outs, axis=1), res.exec_time_ns, res


# revision 41
# speedup vs baseline: 2.3237x; 2.3237x over previous
"""Self-contained Trainium2 Bass kernel for a batched (time-stepped) GAT layer.

Problem: x [N=20000, T=8, F=128], edge_index [2, E=320000] (+self loops),
W [128, 256] (4 heads x 64), att_src/att_dst [4, 64], bias [64].
Per time step: GATConv (concat=False -> head mean) with softmax attention.
Output: [N, T, 64] f32.

Sharding: data-parallel over the T=8 time steps across 8 NeuronCores
(each step independent given shared weights; weights replicated).

Per-core algorithm (one time step):
  Phase 1 (dense): h = x_t @ W_aug (W augmented with 8 columns so the same
    matmul yields per-node a_src/a_dst attention logits). h is written to an
    HBM side array 'hext' ([N, 384] bf16 rows: 256 bf16 h | 4 f32 a_src | pad)
    and per-node aux rows 'aux' ([N+pad, 128] fp16: 4 a_dst | dst_local | pad).
  Phase 2 (edges, sorted by destination on host, chunked 128/dst-tile):
    - dma_gather hext rows by src  -> h[src], a_src[src]
    - dma_gather aux rows by dst   -> a_dst[dst], dst_local (pad edges hit a
      dummy aux row with a_dst=-1000 so exp(alpha) == 0)
    - alpha = leaky_relu(a_src+a_dst); ex = exp(alpha)   (softmax max-shift is
      unnecessary: |alpha| <= ~10, exp stays in f32 range; softmax invariant)
    - msg = h[src] * ex (broadcast per head) ++ ex columns
    - one-hot(dst_local) matmul accumulates segment sums into PSUM:
      numerator [128, 256] and denominators [128, 4] in one [128, 260] matmul
    - out = (numerator / denom).mean(heads) + bias
"""

import numpy as np
import ml_dtypes
from contextlib import ExitStack

import concourse.bass as bass
import concourse.bacc as bacc
import concourse.mybir as mybir
import concourse.tile as tile
from concourse import library_config
from concourse.bass_utils import run_bass_kernel_spmd

F32 = mybir.dt.float32
BF16 = mybir.dt.bfloat16
FP16 = mybir.dt.float16
I16 = mybir.dt.int16

P = 128


class GatConfig:
    def __init__(self, n_nodes, in_dim, heads, d_model, neg_slope):
        self.n_nodes = n_nodes
        self.in_dim = in_dim
        self.heads = heads
        self.d_model = d_model
        self.hc = heads * d_model
        self.neg_slope = neg_slope
        self.n_tiles = (n_nodes + P - 1) // P
        self.n_pad = self.n_tiles * P
        self.dummy_row = self.n_pad            # aux dummy row index
        self.aux_rows = self.n_pad + P
        self.hext_w = self.hc + P              # bf16 cols: hc h | 8 a_src | pad
        self.aux_w = P                         # fp16 cols: heads a_dst | 1 dstloc
        self.mm_w = self.hc + heads            # matmul rhs width (msg | ex)


CFG = GatConfig(n_nodes=20000, in_dim=128, heads=4, d_model=64, neg_slope=0.2)
T_STEPS = 8
N_CORES = 8


def preprocess_edges(cfg, edge_index):
    """Sort (edges + self loops) by destination, pad each 128-node dst tile's
    edge list to a multiple of 128, and produce wrapped int16 gather indices.

    Returns (g1_wrapped, g2_wrapped, chunks_per_tile).
    g1: source-node index per edge slot (pad slots -> 0, harmless: ex==0).
    g2: dst-node index per edge slot (pad slots -> dummy aux row).
    Wrapped layout: flat slot j lives at [j % 16, j // 16], replicated to
    128 partitions (8 copies of the 16-partition group) as HW requires.
    """
    n = cfg.n_nodes
    loops = np.arange(n, dtype=np.int64)
    src = np.concatenate([np.asarray(edge_index[0], dtype=np.int64), loops])
    dst = np.concatenate([np.asarray(edge_index[1], dtype=np.int64), loops])
    order = np.argsort(dst, kind="stable")
    src_s = src[order]
    dst_s = dst[order]
    counts = np.bincount(dst_s // P, minlength=cfg.n_tiles)
    g1_parts, g2_parts, chunks = [], [], []
    pos = 0
    for m in range(cfg.n_tiles):
        length = int(counts[m])
        lpad = max(P, ((length + P - 1) // P) * P)
        g1 = np.zeros(lpad, np.int16)
        g2 = np.full(lpad, cfg.dummy_row, np.int16)
        g1[:length] = src_s[pos : pos + length]
        g2[:length] = dst_s[pos : pos + length]
        g1_parts.append(g1)
        g2_parts.append(g2)
        chunks.append(lpad // P)
        pos += length
    assert pos == src_s.size

    def wrap(flat):
        w = flat.reshape(-1, 16).T.copy()       # [16, E_pad/16]
        return np.tile(w, (8, 1)).copy()        # [128, E_pad/16]

    g1_all = np.concatenate(g1_parts)
    g2_all = np.concatenate(g2_parts)
    # dst_local per edge slot, laid out [128 lanes, chunk]: pad slots get 200
    # (matches no one-hot row -> padded edges contribute nothing).
    dl_flat = np.where(
        g2_all == cfg.dummy_row, 200.0, (g2_all.astype(np.int64) % P).astype(np.float64)
    )
    dl_all = dl_flat.reshape(-1, P).T.astype(ml_dtypes.bfloat16).copy()
    return wrap(g1_all), wrap(g2_all), chunks, dl_all


def build_consts(cfg, W, att_src, att_dst, bias):
    """Host-side constant tensors shared by all cores."""
    W = np.asarray(W, np.float32)
    att_src = np.asarray(att_src, np.float32)
    att_dst = np.asarray(att_dst, np.float32)
    bias = np.asarray(bias, np.float32)
    Wr = W.reshape(cfg.in_dim, cfg.heads, cfg.d_model)
    a_src_cols = np.einsum("fhc,hc->fh", Wr, att_src)
    a_dst_cols = np.einsum("fhc,hc->fh", Wr, att_dst)
    # h channels stored (c, h)-major: col = c*H + h. Keeps the per-head ex
    # broadcast AP's innermost step at 1 (DVE 2x-mode packable).
    W_perm = np.ascontiguousarray(
        Wr.transpose(0, 2, 1).reshape(cfg.in_dim, cfg.hc)
    )
    waug = np.concatenate([W_perm, a_src_cols, a_dst_cols], axis=1)
    biasrep = np.tile(bias[None, :], (P, 1)).astype(np.float32)
    t2row = np.tile(
        np.arange(P, dtype=ml_dtypes.bfloat16)[None, :], (P, 1)
    ).copy()
    pcol = np.arange(P, dtype=np.float16)[:, None].copy()
    ident = np.eye(P, dtype=np.float32)
    return {
        "waug": np.ascontiguousarray(waug, np.float32),
        "biasrep": biasrep,
        "t2row": t2row,
        "pcol": pcol,
        "ident": ident,
    }


def build_nc(cfg, chunks, e16, debug=False, num_devices=N_CORES):
    """Build the full Bass program (SPMD: identical across cores)."""
    nc = bacc.Bacc(
        "TRN2",
        target_bir_lowering=False,
        debug=debug,
        num_devices=num_devices,
        num_swdge_queues=4,
    )
    n_chunks_tot = sum(chunks)
    heads, hc = cfg.heads, cfg.hc
    naug = hc + 2 * heads

    xt = nc.dram_tensor("xt", [cfg.n_nodes, cfg.in_dim], F32, kind="ExternalInput")
    waug = nc.dram_tensor("waug", [cfg.in_dim, naug], F32, kind="ExternalInput")
    biasrep = nc.dram_tensor("biasrep", [P, cfg.d_model], F32, kind="ExternalInput")
    t2row = nc.dram_tensor("t2row", [P, P], BF16, kind="ExternalInput")
    pcol = nc.dram_tensor("pcol", [P, 1], FP16, kind="ExternalInput")
    dl = nc.dram_tensor("dl", [P, n_chunks_tot], BF16, kind="ExternalInput")
    ident = nc.dram_tensor("ident", [P, P], F32, kind="ExternalInput")
    g1 = nc.dram_tensor("g1", [P, e16], I16, kind="ExternalInput")
    g2 = nc.dram_tensor("g2", [P, e16], I16, kind="ExternalInput")
    hext = nc.dram_tensor("hext", [cfg.n_pad, cfg.hext_w], BF16, kind="Internal")
    aux = nc.dram_tensor("aux", [cfg.aux_rows, cfg.aux_w], FP16, kind="Internal")
    out = nc.dram_tensor("out", [cfg.n_nodes, cfg.d_model], F32, kind="ExternalOutput")

    with tile.TileContext(nc) as tc, ExitStack() as ctx:
        nc.gpsimd.load_library(library_config.mlp)
        tc.no_sync_barrier()

        consts = ctx.enter_context(tc.tile_pool(name="consts", bufs=1))
        waug_t = consts.tile([P, naug], F32)
        nc.sync.dma_start(waug_t[:], waug[:, :])
        bias_t = consts.tile([P, cfg.d_model], F32)
        nc.sync.dma_start(bias_t[:], biasrep[:, :])
        t2_t = consts.tile([P, P], BF16)
        nc.sync.dma_start(t2_t[:], t2row[:, :])
        pcol_t = consts.tile([P, 1], FP16)
        nc.sync.dma_start(pcol_t[:], pcol[:, :])
        id_t = consts.tile([P, P], F32)
        nc.sync.dma_start(id_t[:], ident[:, :])

        # Index/const streams are independent of phase 1 — load them up front
        # so they overlap the dense phase instead of stalling behind the
        # barrier.
        idxpool = ctx.enter_context(tc.tile_pool(name="idx", bufs=1))
        g1s = idxpool.tile([P, e16], I16)
        nc.sync.dma_start(g1s[:], g1[:, :])
        g2s = idxpool.tile([P, e16], I16)
        nc.sync.dma_start(g2s[:], g2[:, :])
        dls = idxpool.tile([P, n_chunks_tot], BF16)
        nc.sync.dma_start(dls[:], dl[:, :])

        # ---------------- phase 1: dense h + logits ----------------
        h_scope = nc.enter_named_scope("h_phase", False)[0]
        xpool = ctx.enter_context(tc.tile_pool(name="x", bufs=3))
        stpool = ctx.enter_context(tc.tile_pool(name="stage", bufs=3))
        ps_tr = ctx.enter_context(tc.tile_pool(name="ps_tr", bufs=2, space="PSUM"))
        ps_h = ctx.enter_context(tc.tile_pool(name="ps_h", bufs=2, space="PSUM"))

        for m in range(cfg.n_tiles):
            n0 = m * P
            nrows = min(P, cfg.n_nodes - n0)
            xtile = xpool.tile([P, cfg.in_dim], F32, tag="xtile")
            if nrows < P:
                nc.vector.memset(xtile[:], 0.0)
            nc.sync.dma_start(xtile[:nrows, :], xt[n0 : n0 + nrows, :])
            ptr = ps_tr.tile([P, P], F32)
            nc.tensor.transpose(ptr[:], xtile[:], id_t[:])
            xT = xpool.tile([P, P], F32, tag="xT")
            nc.vector.tensor_copy(xT[:], ptr[:])
            ph = ps_h.tile([P, naug], F32)
            nc.tensor.matmul(ph[:], xT[:], waug_t[:], start=True, stop=True)

            stage = stpool.tile([P, cfg.hext_w], BF16, tag="stage")
            nc.vector.memset(stage[:, hc + 8 :], 0.0)
            nc.vector.tensor_copy(stage[:, 0:hc], ph[:, 0:hc])
            nc.vector.tensor_copy(
                stage[:, hc : hc + 8].bitcast(F32), ph[:, hc : hc + heads]
            )
            nc.sync.dma_start(hext[n0 : n0 + P, :], stage[:])

            astage = stpool.tile([P, cfg.aux_w], FP16, tag="astage")
            nc.vector.memset(astage[:], 0.0)
            nc.vector.tensor_copy(astage[:, 0:heads], ph[:, hc + heads : naug])
            nc.vector.tensor_copy(astage[:, heads : heads + 1], pcol_t[:])
            nc.sync.dma_start(aux[n0 : n0 + P, :], astage[:])

        # dummy aux rows for padded edge slots: a_dst = -1000 => ex == 0
        dstage = stpool.tile([P, cfg.aux_w], FP16, tag="astage")
        nc.vector.memset(dstage[:], -1000.0)
        nc.sync.dma_start(aux[cfg.n_pad : cfg.n_pad + P, :], dstage[:])

        nc.leave_named_scope("h_phase", h_scope, False)
        tc.strict_bb_all_engine_barrier()

        # ---------------- phase 2: edge message passing ----------------
        e_scope = nc.enter_named_scope("edge_phase", False)[0]

        max_ch = max(chunks)
        hpool = ctx.enter_context(tc.tile_pool(name="hrow", bufs=2))
        apool = ctx.enter_context(tc.tile_pool(name="arow", bufs=2))
        mpool = ctx.enter_context(tc.tile_pool(name="msg", bufs=2))
        ohpool = ctx.enter_context(tc.tile_pool(name="oh", bufs=2))
        spool = ctx.enter_context(tc.tile_pool(name="small", bufs=2))
        ps_e = ctx.enter_context(tc.tile_pool(name="ps_e", bufs=2, space="PSUM"))

        # Split gathers: SWDGE descriptor-ring carveout holds ~256 descs per
        # partition; one gather emits num_idxs/16 descs per partition, so keep
        # each call at <= GMAX indices.
        GMAX_CH = 8  # 1024 indices / call
        sub_lens = set()
        for nch in set(chunks):
            for c0 in range(0, nch, GMAX_CH):
                sub_lens.add(min(GMAX_CH, nch - c0) * P)
        lregs = {l: nc.gpsimd.to_reg(l) for l in sorted(sub_lens)}

        def next_q():
            # queue_num is rewritten post-scheduling (see below) to match the
            # DMASW sem lane Tile assigned; sem lanes can't span queues.
            return 0

        off = 0
        chunk_base = 0
        for m in range(cfg.n_tiles):
            nch = chunks[m]
            hrow = hpool.tile([P, max_ch, cfg.hext_w], BF16)
            arow = apool.tile([P, max_ch, cfg.aux_w], FP16)
            for c0 in range(0, nch, GMAX_CH):
                cc = min(GMAX_CH, nch - c0)
                ll = cc * P
                o0 = off + c0 * P
                nc.gpsimd.dma_gather(
                    hrow[:, c0 : c0 + cc, :],
                    hext[:, :],
                    g1s[:, o0 // 16 : (o0 + ll) // 16],
                    ll,
                    lregs[ll],
                    cfg.hext_w,
                    queue_num=next_q(),
                )
                nc.gpsimd.dma_gather(
                    arow[:, c0 : c0 + cc, :],
                    aux[:, :],
                    g2s[:, o0 // 16 : (o0 + ll) // 16],
                    ll,
                    lregs[ll],
                    cfg.aux_w,
                    queue_num=next_q(),
                )
            off += nch * P

            # alpha = leaky_relu(a_src[src] + a_dst[dst]); ex = exp(alpha)
            alpha = spool.tile([P, max_ch, heads], F32, tag="alpha")
            nc.vector.tensor_add(
                alpha[:, 0:nch, :],
                hrow[:, 0:nch, hc : hc + 8].bitcast(F32),
                arow[:, 0:nch, 0:heads],
            )
            lrt = spool.tile([P, max_ch, heads], F32, tag="lrt")
            nc.vector.tensor_scalar_mul(lrt[:, 0:nch, :], alpha[:, 0:nch, :], cfg.neg_slope)
            nc.vector.tensor_max(alpha[:, 0:nch, :], alpha[:, 0:nch, :], lrt[:, 0:nch, :])
            exb = spool.tile([P, max_ch, heads], BF16, tag="exb")
            nc.scalar.activation(
                exb[:, 0:nch, :], alpha[:, 0:nch, :], mybir.ActivationFunctionType.Exp
            )

            # weighted messages: msg[:, :, :hc] = h * ex (per head), ++ ex cols.
            # Channels are (c, h)-major so the ex broadcast has innermost
            # step 1 (packable -> DVE 2x mode).
            msg = mpool.tile([P, max_ch, cfg.mm_w], BF16)
            nc.vector.tensor_tensor(
                msg[:, 0:nch, 0:hc].rearrange("p n (c h) -> p n c h", h=heads),
                hrow[:, 0:nch, 0:hc].rearrange("p n (c h) -> p n c h", h=heads),
                exb[:, 0:nch, :]
                .rearrange("p n h -> p n () h")
                .broadcast_to((P, nch, cfg.d_model, heads)),
                op=mybir.AluOpType.mult,
            )
            nc.vector.tensor_copy(msg[:, 0:nch, hc : cfg.mm_w], exb[:, 0:nch, :])

            # one-hot(dst_local) for all chunks of the tile in one DVE op
            oh_all = ohpool.tile([P, max_ch, P], BF16)
            nc.vector.tensor_tensor(
                oh_all[:, 0:nch, :],
                t2_t[:].rearrange("p d -> p () d").broadcast_to((P, nch, P)),
                dls[:, chunk_base : chunk_base + nch]
                .rearrange("p n -> p n ()")
                .broadcast_to((P, nch, P)),
                op=mybir.AluOpType.is_equal,
            )

            # segment sums via one-hot matmul into PSUM
            pe = ps_e.tile([P, cfg.mm_w], F32)
            for ch in range(nch):
                nc.tensor.matmul(
                    pe[:],
                    oh_all[:, ch, :],
                    msg[:, ch, :],
                    start=(ch == 0),
                    stop=(ch == nch - 1),
                )

            # out = (numerator / denom).mean(heads) + bias
            r = spool.tile([P, heads], F32, tag="r")
            nc.vector.reciprocal(r[:], pe[:, hc : cfg.mm_w])
            nc.vector.tensor_scalar_mul(r[:], r[:], 1.0 / heads)
            wm = spool.tile([P, cfg.d_model, heads], F32, tag="wm")
            nc.vector.tensor_tensor(
                wm[:],
                pe[:, 0:hc].rearrange("p (c h) -> p c h", h=heads),
                r[:].rearrange("p h -> p () h").broadcast_to((P, cfg.d_model, heads)),
                op=mybir.AluOpType.mult,
            )
            onode = spool.tile([P, cfg.d_model], F32, tag="onode")
            nc.vector.tensor_reduce(
                onode[:],
                wm[:],
                axis=mybir.AxisListType.X,
                op=mybir.AluOpType.add,
            )
            nc.vector.tensor_add(onode[:], onode[:], bias_t[:])
            n0 = m * P
            nrows = min(P, cfg.n_nodes - n0)
            nc.sync.dma_start(out[n0 : n0 + nrows, :], onode[:nrows, :])
            chunk_base += nch

        nc.leave_named_scope("edge_phase", e_scope, False)

    # Spread gathers over the 4 SWDGE queues. Each DMASW sem lane is locked to
    # one queue, so derive the queue from the lane Tile assigned (k % 4).
    import re

    for f in nc.m.functions:
        for bb in f.blocks:
            for inst in bb.instructions:
                if isinstance(inst, mybir.InstDMAGatherAnt):
                    si = inst.sync_info
                    if si and si.on_update:
                        name = getattr(si.on_update[0], "ant_name", "") or ""
                        mt = re.match(r"DMASW(\d+)", name)
                        if mt:
                            inst.queue_num = int(mt.group(1)) % 4

    nc.compile()
    return nc


_CACHE = {}


def _prepare(x, edge_index, W, att_src, att_dst, bias):
    cfg = CFG
    x = np.asarray(x, np.float32)
    key = hash(np.asarray(edge_index).tobytes())
    if key not in _CACHE:
        g1w, g2w, chunks, dl_all = preprocess_edges(cfg, edge_index)
        nc = build_nc(cfg, chunks, g1w.shape[1], debug=False, num_devices=N_CORES)
        _CACHE.clear()
        _CACHE[key] = (nc, g1w, g2w, dl_all)
    nc, g1w, g2w, dl_all = _CACHE[key]
    consts = build_consts(cfg, W, att_src, att_dst, bias)
    in_maps = []
    for t in range(T_STEPS):
        in_maps.append(
            {
                "xt": np.ascontiguousarray(x[:, t, :]),
                "g1": g1w,
                "g2": g2w,
                "dl": dl_all,
                **consts,
            }
        )
    return nc, in_maps


def kernel(x, edge_index, W, att_src, att_dst, bias):
    nc, in_maps = _prepare(x, edge_index, W, att_src, att_dst, bias)
    res = run_bass_kernel_spmd(nc, in_maps, core_ids=list(range(N_CORES)))
    outs = [res.results[t]["out"] for t in range(T_STEPS)]
    return np.stack(outs, axis=1)  # [N, T, C]


def kernel_profiled(x, edge_index, W, att_src, att_dst, bias):
    """Run with NTFF tracing; returns (output, exec_time_ns, results obj)."""
    nc, in_maps = _prepare(x, edge_index, W, att_src, att_dst, bias)
    res = run_bass_kernel_spmd(
        nc, in_maps, core_ids=list(range(N_CORES)), trace=True
    )
    outs = [res.results[t]["out"] for t in range(T_STEPS)]
    return np.stack(outs, axis=1), res.exec_time_ns, res


# revision 49
# speedup vs baseline: 2.6682x; 1.1483x over previous
"""Self-contained Trainium2 Bass kernel for a batched (time-stepped) GAT layer.

Problem: x [N=20000, T=8, F=128], edge_index [2, E=320000] (+self loops),
W [128, 256] (4 heads x 64), att_src/att_dst [4, 64], bias [64].
Per time step: GATConv (concat=False -> head mean) with softmax attention.
Output: [N, T, 64] f32.

Sharding: data-parallel over the T=8 time steps across 8 NeuronCores
(each step independent given shared weights; weights replicated).

Per-core algorithm (one time step):
  Phase 1 (dense): h = x_t @ W_aug (W augmented with 8 columns so the same
    matmul yields per-node a_src/a_dst attention logits). h is written to an
    HBM side array 'hext' ([N, 384] bf16 rows: 256 bf16 h | 4 f32 a_src | pad)
    rows also carry a_dst; a 256B tail slice of the same rows serves the
    dst-indexed gather).
  Phase 2 (edges, sorted by destination on host, chunked 128/dst-tile):
    - dma_gather hext rows by src  -> h[src], a_src[src]
    - dma_gather hext tail slices by dst -> a_dst[dst] (pad edges hit a
      dummy row with a_dst=-1000 so exp(alpha) == 0)
    - alpha = leaky_relu(a_src+a_dst); ex = exp(alpha)   (softmax max-shift is
      unnecessary: |alpha| <= ~10, exp stays in f32 range; softmax invariant)
    - msg = h[src] * ex (broadcast per head) ++ ex columns
    - one-hot(dst_local) matmul accumulates segment sums into PSUM:
      numerator [128, 256] and denominators [128, 4] in one [128, 260] matmul
    - out = (numerator / denom).mean(heads) + bias
"""

import numpy as np
import ml_dtypes
from contextlib import ExitStack

import concourse.bass as bass
import concourse.bacc as bacc
import concourse.mybir as mybir
import concourse.tile as tile
from concourse import library_config
from concourse.bass_utils import run_bass_kernel_spmd

F32 = mybir.dt.float32
BF16 = mybir.dt.bfloat16
FP16 = mybir.dt.float16
I16 = mybir.dt.int16

P = 128


class GatConfig:
    def __init__(self, n_nodes, in_dim, heads, d_model, neg_slope):
        self.n_nodes = n_nodes
        self.in_dim = in_dim
        self.heads = heads
        self.d_model = d_model
        self.hc = heads * d_model
        self.neg_slope = neg_slope
        self.n_tiles = (n_nodes + P - 1) // P
        self.n_pad = self.n_tiles * P
        self.dummy_row = self.n_pad            # dummy hext row for padded edges
        self.hext_rows = self.n_pad + P
        # bf16 cols: hc h | 8 (a_src f32) | 8 (a_dst f32) | pad to hc+128
        self.hext_w = self.hc + P
        self.aux_w = P                         # gather2: bf16 cols hc..hc+128
        self.mm_w = self.hc + heads            # matmul rhs width (msg | ex)


CFG = GatConfig(n_nodes=20000, in_dim=128, heads=4, d_model=64, neg_slope=0.2)
T_STEPS = 8
N_CORES = 8


def preprocess_edges(cfg, edge_index):
    """Sort (edges + self loops) by destination, pad each 128-node dst tile's
    edge list to a multiple of 128, and produce wrapped int16 gather indices.

    Returns (g1_wrapped, g2_wrapped, chunks_per_tile).
    g1: source-node index per edge slot (pad slots -> 0, harmless: ex==0).
    g2: dst-node index per edge slot (pad slots -> dummy aux row).
    Wrapped layout: flat slot j lives at [j % 16, j // 16], replicated to
    128 partitions (8 copies of the 16-partition group) as HW requires.
    """
    n = cfg.n_nodes
    loops = np.arange(n, dtype=np.int64)
    src = np.concatenate([np.asarray(edge_index[0], dtype=np.int64), loops])
    dst = np.concatenate([np.asarray(edge_index[1], dtype=np.int64), loops])
    order = np.argsort(dst, kind="stable")
    src_s = src[order]
    dst_s = dst[order]
    counts = np.bincount(dst_s // P, minlength=cfg.n_tiles)
    g1_parts, g2_parts, chunks = [], [], []
    pos = 0
    for m in range(cfg.n_tiles):
        length = int(counts[m])
        lpad = max(P, ((length + P - 1) // P) * P)
        g1 = np.zeros(lpad, np.int16)
        g2 = np.full(lpad, cfg.dummy_row, np.int16)
        g1[:length] = src_s[pos : pos + length]
        g2[:length] = dst_s[pos : pos + length]
        g1_parts.append(g1)
        g2_parts.append(g2)
        chunks.append(lpad // P)
        pos += length
    assert pos == src_s.size

    def wrap(flat):
        w = flat.reshape(-1, 16).T.copy()       # [16, E_pad/16]
        return np.tile(w, (8, 1)).copy()        # [128, E_pad/16]

    g1_all = np.concatenate(g1_parts)
    g2_all = np.concatenate(g2_parts)
    # dst_local per edge slot, laid out [128 lanes, chunk]: pad slots get 200
    # (matches no one-hot row -> padded edges contribute nothing).
    dl_flat = np.where(
        g2_all == cfg.dummy_row, 200.0, (g2_all.astype(np.int64) % P).astype(np.float64)
    )
    dl_all = dl_flat.reshape(-1, P).T.astype(ml_dtypes.bfloat16).copy()
    return wrap(g1_all), wrap(g2_all), chunks, dl_all


def build_consts(cfg, W, att_src, att_dst, bias):
    """Host-side constant tensors shared by all cores."""
    W = np.asarray(W, np.float32)
    att_src = np.asarray(att_src, np.float32)
    att_dst = np.asarray(att_dst, np.float32)
    bias = np.asarray(bias, np.float32)
    Wr = W.reshape(cfg.in_dim, cfg.heads, cfg.d_model)
    a_src_cols = np.einsum("fhc,hc->fh", Wr, att_src)
    a_dst_cols = np.einsum("fhc,hc->fh", Wr, att_dst)
    # h channels stored (c, h)-major: col = c*H + h. Keeps the per-head ex
    # broadcast AP's innermost step at 1 (DVE 2x-mode packable).
    W_perm = np.ascontiguousarray(
        Wr.transpose(0, 2, 1).reshape(cfg.in_dim, cfg.hc)
    )
    waug = np.concatenate([W_perm, a_src_cols, a_dst_cols], axis=1)
    biasrep = np.tile(bias[None, :], (P, 1)).astype(np.float32)
    t2row = np.tile(
        np.arange(P, dtype=ml_dtypes.bfloat16)[None, :], (P, 1)
    ).copy()
    ident = np.eye(P, dtype=np.float32)
    return {
        "waug": np.ascontiguousarray(waug, np.float32),
        "biasrep": biasrep,
        "t2row": t2row,
        "ident": ident,
    }


def build_nc(cfg, chunks, e16, debug=False, num_devices=N_CORES):
    """Build the full Bass program (SPMD: identical across cores)."""
    nc = bacc.Bacc(
        "TRN2",
        target_bir_lowering=False,
        debug=debug,
        num_devices=num_devices,
        num_swdge_queues=4,
    )
    n_chunks_tot = sum(chunks)
    heads, hc = cfg.heads, cfg.hc
    naug = hc + 2 * heads

    xt = nc.dram_tensor("xt", [cfg.n_nodes, cfg.in_dim], F32, kind="ExternalInput")
    waug = nc.dram_tensor("waug", [cfg.in_dim, naug], F32, kind="ExternalInput")
    biasrep = nc.dram_tensor("biasrep", [P, cfg.d_model], F32, kind="ExternalInput")
    t2row = nc.dram_tensor("t2row", [P, P], BF16, kind="ExternalInput")
    dl = nc.dram_tensor("dl", [P, n_chunks_tot], BF16, kind="ExternalInput")
    ident = nc.dram_tensor("ident", [P, P], F32, kind="ExternalInput")
    g1 = nc.dram_tensor("g1", [P, e16], I16, kind="ExternalInput")
    g2 = nc.dram_tensor("g2", [P, e16], I16, kind="ExternalInput")
    hext = nc.dram_tensor("hext", [cfg.hext_rows, cfg.hext_w], BF16, kind="Internal")
    out = nc.dram_tensor("out", [cfg.n_nodes, cfg.d_model], F32, kind="ExternalOutput")

    with tile.TileContext(nc) as tc, ExitStack() as ctx:
        nc.gpsimd.load_library(library_config.mlp)
        tc.no_sync_barrier()

        consts = ctx.enter_context(tc.tile_pool(name="consts", bufs=1))
        waug_t = consts.tile([P, naug], F32)
        nc.sync.dma_start(waug_t[:], waug[:, :])
        bias_t = consts.tile([P, cfg.d_model], F32)
        nc.sync.dma_start(bias_t[:], biasrep[:, :])
        t2_t = consts.tile([P, P], BF16)
        nc.sync.dma_start(t2_t[:], t2row[:, :])
        id_t = consts.tile([P, P], F32)
        nc.sync.dma_start(id_t[:], ident[:, :])

        # dst_local constants are tiny — keep resident; gather indices are
        # streamed per tile (88KB resident would crowd out double-buffering).
        idxpool = ctx.enter_context(tc.tile_pool(name="idx", bufs=1))
        dls = idxpool.tile([P, n_chunks_tot], BF16)
        nc.sync.dma_start(dls[:], dl[:, :])

        # ---------------- phase 1: dense h + logits ----------------
        h_scope = nc.enter_named_scope("h_phase", False)[0]
        xpool = ctx.enter_context(tc.tile_pool(name="x", bufs=3))
        stpool = ctx.enter_context(tc.tile_pool(name="stage", bufs=3))
        ps_tr = ctx.enter_context(tc.tile_pool(name="ps_tr", bufs=2, space="PSUM"))
        ps_h = ctx.enter_context(tc.tile_pool(name="ps_h", bufs=2, space="PSUM"))

        for m in range(cfg.n_tiles):
            n0 = m * P
            nrows = min(P, cfg.n_nodes - n0)
            xtile = xpool.tile([P, cfg.in_dim], F32, tag="xtile")
            if nrows < P:
                nc.vector.memset(xtile[:], 0.0)
            nc.sync.dma_start(xtile[:nrows, :], xt[n0 : n0 + nrows, :])
            ptr = ps_tr.tile([P, P], F32)
            nc.tensor.transpose(ptr[:], xtile[:], id_t[:])
            xT = xpool.tile([P, P], F32, tag="xT")
            nc.vector.tensor_copy(xT[:], ptr[:])
            ph = ps_h.tile([P, naug], F32)
            nc.tensor.matmul(ph[:], xT[:], waug_t[:], start=True, stop=True)

            stage = stpool.tile([P, cfg.hext_w], BF16, tag="stage")
            nc.vector.memset(stage[:, hc + 16 :], 0.0)
            nc.vector.tensor_copy(stage[:, 0:hc], ph[:, 0:hc])
            nc.vector.tensor_copy(
                stage[:, hc : hc + 16].bitcast(F32), ph[:, hc:naug]
            )
            nc.sync.dma_start(hext[n0 : n0 + P, :], stage[:])

        # dummy hext rows for padded edge slots: a_dst = -1000 => ex == 0
        dstage = stpool.tile([P, cfg.hext_w], BF16, tag="stage")
        nc.vector.memset(dstage[:], 0.0)
        nc.vector.memset(dstage[:, hc : hc + 16].bitcast(F32), -1000.0)
        nc.sync.dma_start(hext[cfg.n_pad : cfg.n_pad + P, :], dstage[:])

        nc.leave_named_scope("h_phase", h_scope, False)
        tc.strict_bb_all_engine_barrier()

        # ---------------- phase 2: edge message passing ----------------
        e_scope = nc.enter_named_scope("edge_phase", False)[0]

        max_ch = max(chunks)
        hpool = ctx.enter_context(tc.tile_pool(name="hrow", bufs=3))
        apool = ctx.enter_context(tc.tile_pool(name="arow", bufs=3))
        mpool = ctx.enter_context(tc.tile_pool(name="msg", bufs=3))
        ohpool = ctx.enter_context(tc.tile_pool(name="oh", bufs=3))
        spool = ctx.enter_context(tc.tile_pool(name="small", bufs=3))
        gpool = ctx.enter_context(tc.tile_pool(name="gidx", bufs=3))
        ps_e = ctx.enter_context(tc.tile_pool(name="ps_e", bufs=4, space="PSUM"))

        # Split gathers: SWDGE descriptor-ring carveout holds ~256 descs per
        # partition; one gather emits num_idxs/16 descs per partition, so keep
        # each call at <= GMAX indices.
        GMAX_CH = 8  # 1024 indices / call
        sub_lens = set()
        for nch in set(chunks):
            for c0 in range(0, nch, GMAX_CH):
                sub_lens.add(min(GMAX_CH, nch - c0) * P)
        lregs = {l: nc.gpsimd.to_reg(l) for l in sorted(sub_lens)}

        def next_q():
            # queue_num is rewritten post-scheduling (see below) to match the
            # DMASW sem lane Tile assigned; sem lanes can't span queues.
            return 0

        off = 0
        chunk_base = 0
        for m in range(cfg.n_tiles):
            nch = chunks[m]
            L16 = nch * P // 16
            g1t = gpool.tile([P, (max_ch * P) // 16], I16, tag="g1t")
            nc.sync.dma_start(g1t[:, 0:L16], g1[:, off // 16 : off // 16 + L16])
            g2t = gpool.tile([P, (max_ch * P) // 16], I16, tag="g2t")
            nc.sync.dma_start(g2t[:, 0:L16], g2[:, off // 16 : off // 16 + L16])
            hrow = hpool.tile([P, max_ch, cfg.hext_w], BF16)
            arow = apool.tile([P, max_ch, cfg.aux_w], BF16)
            for c0 in range(0, nch, GMAX_CH):
                cc = min(GMAX_CH, nch - c0)
                ll = cc * P
                o0 = c0 * P
                nc.gpsimd.dma_gather(
                    hrow[:, c0 : c0 + cc, :],
                    hext[:, :],
                    g1t[:, o0 // 16 : (o0 + ll) // 16],
                    ll,
                    lregs[ll],
                    cfg.hext_w,
                    queue_num=next_q(),
                )
                nc.gpsimd.dma_gather(
                    arow[:, c0 : c0 + cc, :],
                    hext[:, hc : hc + P],
                    g2t[:, o0 // 16 : (o0 + ll) // 16],
                    ll,
                    lregs[ll],
                    cfg.aux_w,
                    elem_step=cfg.hext_w,
                    queue_num=next_q(),
                )
            off += nch * P

            # alpha = leaky_relu(a_src[src] + a_dst[dst]); ex = exp(alpha)
            alpha = spool.tile([P, max_ch, heads], F32, tag="alpha")
            nc.vector.tensor_add(
                alpha[:, 0:nch, :],
                hrow[:, 0:nch, hc : hc + 8].bitcast(F32),
                arow[:, 0:nch, 8:16].bitcast(F32),
            )
            lrt = spool.tile([P, max_ch, heads], F32, tag="lrt")
            nc.vector.tensor_scalar_mul(lrt[:, 0:nch, :], alpha[:, 0:nch, :], cfg.neg_slope)
            nc.vector.tensor_max(alpha[:, 0:nch, :], alpha[:, 0:nch, :], lrt[:, 0:nch, :])
            exb = spool.tile([P, max_ch, heads], BF16, tag="exb")
            nc.scalar.activation(
                exb[:, 0:nch, :], alpha[:, 0:nch, :], mybir.ActivationFunctionType.Exp
            )

            # weighted messages: msg[:, :, :hc] = h * ex (per head), ++ ex cols.
            # Channels are (c, h)-major so the ex broadcast has innermost
            # step 1 (packable -> DVE 2x mode).
            msg = mpool.tile([P, max_ch, cfg.mm_w], BF16)
            nc.vector.tensor_tensor(
                msg[:, 0:nch, 0:hc].rearrange("p n (c h) -> p n c h", h=heads),
                hrow[:, 0:nch, 0:hc].rearrange("p n (c h) -> p n c h", h=heads),
                exb[:, 0:nch, :]
                .rearrange("p n h -> p n () h")
                .broadcast_to((P, nch, cfg.d_model, heads)),
                op=mybir.AluOpType.mult,
            )
            nc.vector.tensor_copy(msg[:, 0:nch, hc : cfg.mm_w], exb[:, 0:nch, :])

            # one-hot(dst_local) for all chunks of the tile in one DVE op
            oh_all = ohpool.tile([P, max_ch, P], BF16)
            nc.vector.tensor_tensor(
                oh_all[:, 0:nch, :],
                t2_t[:].rearrange("p d -> p () d").broadcast_to((P, nch, P)),
                dls[:, chunk_base : chunk_base + nch]
                .rearrange("p n -> p n ()")
                .broadcast_to((P, nch, P)),
                op=mybir.AluOpType.is_equal,
            )

            # segment sums via one-hot matmul into PSUM
            pe = ps_e.tile([P, cfg.mm_w], F32)
            for ch in range(nch):
                nc.tensor.matmul(
                    pe[:],
                    oh_all[:, ch, :],
                    msg[:, ch, :],
                    start=(ch == 0),
                    stop=(ch == nch - 1),
                )

            # out = (numerator / denom).mean(heads) + bias
            r = spool.tile([P, heads], F32, tag="r")
            nc.vector.reciprocal(r[:], pe[:, hc : cfg.mm_w])
            nc.vector.tensor_scalar_mul(r[:], r[:], 1.0 / heads)
            wm = spool.tile([P, cfg.d_model, heads], F32, tag="wm")
            nc.vector.tensor_tensor(
                wm[:],
                pe[:, 0:hc].rearrange("p (c h) -> p c h", h=heads),
                r[:].rearrange("p h -> p () h").broadcast_to((P, cfg.d_model, heads)),
                op=mybir.AluOpType.mult,
            )
            onode = spool.tile([P, cfg.d_model], F32, tag="onode")
            nc.vector.tensor_reduce(
                onode[:],
                wm[:],
                axis=mybir.AxisListType.X,
                op=mybir.AluOpType.add,
            )
            nc.vector.tensor_add(onode[:], onode[:], bias_t[:])
            n0 = m * P
            nrows = min(P, cfg.n_nodes - n0)
            nc.sync.dma_start(out[n0 : n0 + nrows, :], onode[:nrows, :])
            chunk_base += nch

        nc.leave_named_scope("edge_phase", e_scope, False)

    # Spread gathers over the 4 SWDGE queues. Each DMASW sem lane is locked to
    # one queue, so derive the queue from the lane Tile assigned (k % 4).
    import re

    for f in nc.m.functions:
        for bb in f.blocks:
            for inst in bb.instructions:
                if isinstance(inst, mybir.InstDMAGatherAnt):
                    si = inst.sync_info
                    if si and si.on_update:
                        name = getattr(si.on_update[0], "ant_name", "") or ""
                        mt = re.match(r"DMASW(\d+)", name)
                        if mt:
                            inst.queue_num = int(mt.group(1)) % 4

    nc.compile()
    return nc


_CACHE = {}


def _prepare(x, edge_index, W, att_src, att_dst, bias):
    cfg = CFG
    x = np.asarray(x, np.float32)
    key = hash(np.asarray(edge_index).tobytes())
    if key not in _CACHE:
        g1w, g2w, chunks, dl_all = preprocess_edges(cfg, edge_index)
        nc = build_nc(cfg, chunks, g1w.shape[1], debug=False, num_devices=N_CORES)
        _CACHE.clear()
        _CACHE[key] = (nc, g1w, g2w, dl_all)
    nc, g1w, g2w, dl_all = _CACHE[key]
    consts = build_consts(cfg, W, att_src, att_dst, bias)
    in_maps = []
    for t in range(T_STEPS):
        in_maps.append(
            {
                "xt": np.ascontiguousarray(x[:, t, :]),
                "g1": g1w,
                "g2": g2w,
                "dl": dl_all,
                **consts,
            }
        )
    return nc, in_maps


def kernel(x, edge_index, W, att_src, att_dst, bias):
    nc, in_maps = _prepare(x, edge_index, W, att_src, att_dst, bias)
    res = run_bass_kernel_spmd(nc, in_maps, core_ids=list(range(N_CORES)))
    outs = [res.results[t]["out"] for t in range(T_STEPS)]
    return np.stack(outs, axis=1)  # [N, T, C]


def kernel_profiled(x, edge_index, W, att_src, att_dst, bias):
    """Run with NTFF tracing; returns (output, exec_time_ns, results obj)."""
    nc, in_maps = _prepare(x, edge_index, W, att_src, att_dst, bias)
    res = run_bass_kernel_spmd(
        nc, in_maps, core_ids=list(range(N_CORES)), trace=True
    )
    outs = [res.results[t]["out"] for t in range(T_STEPS)]
    return np.stack(outs, axis=1), res.exec_time_ns, res
